# revision 1
# baseline (speedup 1.0000x reference)
"""Distributed Bass kernel for causal multi-head attention with RoPE.

Problem: B=2, S=2048, D=2048, H=16, HD=128 (nn_Attention_85315230368481).

Sharding: sequence-parallel over 8 cores. Core c owns query rows
[c*256, (c+1)*256) of both batches (512 rows total). Each core projects
Q/K/V for its own rows, applies RoPE to Q and K, AllGathers K^T and V
across cores (bf16), then computes full attention for its query rows
over all keys, and the output projection for its rows. The host
concatenates the 8 row-shards into the full output. No AllReduce.

Layout tricks:
 - x is passed transposed ([D, 512]) so Q^T/K^T ([head_dim, rows]) and
   V (natural [rows, D]) all come straight out of the PE array.
 - Wq/Wk columns are permuted per head (even dims then odd dims) so RoPE
   works on contiguous partition halves; scores are invariant to the
   permutation since both Q and K use it.
 - Scores are computed transposed ([keys, queries]) so exp(scores) is
   directly the moving operand of the attention*V matmul, and the
   softmax denominator is an accumulating ones-column matmul.
 - All matmuls in bf16 (inputs rounded; accumulation stays fp32 in
   PSUM), which runs the PE at full rate with fast weight loads and
   halves every DMA/collective byte count.
"""

import sys

import ml_dtypes
import numpy as np

if "/opt/trn_rl_repo" not in sys.path:
    sys.path.insert(0, "/opt/trn_rl_repo")

B, S, D, H = 2, 2048, 2048, 16
HD = D // H            # 128
NCORES = 8
SQ = S // NCORES       # 256 query rows per core per batch
ROWS = B * SQ          # 512 rows per core
DCH = D // 128         # 16 contraction chunks
SCALE = 1.0 / float(np.sqrt(HD))
BF16 = ml_dtypes.bfloat16

_GRAPH = None
_TRACE = False
_LAST_EXEC_NS = None
_LAST_RES = None


def _build_graph():
    import concourse.mybir as mybir
    from concourse import bacc, tile

    f32 = mybir.dt.float32
    bf = mybir.dt.bfloat16
    Exp = mybir.ActivationFunctionType.Exp

    nc = bacc.Bacc("TRN2", target_bir_lowering=False, num_devices=NCORES)

    xT = nc.declare_dram_parameter("xT", [D, ROWS], bf, isOutput=False)
    wq = nc.declare_dram_parameter("wq", [D, D], bf, isOutput=False)
    wk = nc.declare_dram_parameter("wk", [D, D], bf, isOutput=False)
    wv = nc.declare_dram_parameter("wv", [D, D], bf, isOutput=False)
    wo = nc.declare_dram_parameter("wo", [D, D], bf, isOutput=False)
    cosT = nc.declare_dram_parameter("cosT", [HD // 2, SQ], f32, isOutput=False)
    sinT = nc.declare_dram_parameter("sinT", [HD // 2, SQ], f32, isOutput=False)
    maskT = nc.declare_dram_parameter("maskT", [S, SQ], f32, isOutput=False)
    onesd = nc.declare_dram_parameter("ones", [128, 128], bf, isOutput=False)
    out = nc.declare_dram_parameter("out", [ROWS, D], f32, isOutput=True)

    with nc.allow_low_precision(reason="bf16 matmul inputs; fp32 accumulate"), \
         tile.TileContext(nc) as tc:
        with (
            tc.tile_pool(name="dram", bufs=1, space="DRAM") as dramp,
            tc.tile_pool(name="const", bufs=1) as constp,
            tc.tile_pool(name="wstream", bufs=10) as wpool,
            tc.tile_pool(name="sbout", bufs=4) as sbout,
        ):
            k_in = dramp.tile([D, ROWS], bf)
            k_outs = [
                dramp.tile([NCORES * 512, ROWS], bf, addr_space="Shared",
                           name=f"k_out{i}")
                for i in range(4)
            ]
            v_in = dramp.tile([ROWS, D], bf)
            v_out = dramp.tile([NCORES * ROWS, D], bf, addr_space="Shared")

            # resident tensors
            xts = constp.tile([128, DCH * ROWS], bf)         # x^T chunks
            for d in range(DCH):
                nc.sync.dma_start(
                    out=xts[:, d * ROWS:(d + 1) * ROWS],
                    in_=xT[d * 128:(d + 1) * 128, :],
                )
            cos_sb = constp.tile([64, SQ], f32)
            sin_sb = constp.tile([64, SQ], f32)
            nc.sync.dma_start(out=cos_sb[:], in_=cosT[:, :])
            nc.sync.dma_start(out=sin_sb[:], in_=sinT[:, :])
            mask_sb = constp.tile([128, 16 * SQ], f32)       # maskT chunks
            ones_sq = constp.tile([128, 128], bf)
            nc.sync.dma_start(out=ones_sq[:], in_=onesd[:, :])

            qsb = constp.tile([128, H * ROWS], bf)           # rope'd Q^T per head
            attn = constp.tile([128, H * ROWS], bf)          # attention out^T per head

            def rope(dst, dst_cols, src_ps, tmp_pool):
                # src_ps: [128, ROWS] psum, rows 0:64 = even dims, 64:128 = odd
                for b2 in range(B):
                    cs = slice(b2 * SQ, (b2 + 1) * SQ)
                    dcs = slice(dst_cols + b2 * SQ, dst_cols + (b2 + 1) * SQ)
                    te = src_ps[0:64, cs]
                    to = src_ps[64:128, cs]
                    t1 = tmp_pool.tile([64, SQ], f32, tag="ropetmp1")
                    t2 = tmp_pool.tile([64, SQ], f32, tag="ropetmp2")
                    nc.vector.tensor_mul(t1[:], te, cos_sb[:])
                    nc.vector.tensor_mul(t2[:], to, sin_sb[:])
                    nc.vector.tensor_sub(dst[0:64, dcs], t1[:], t2[:])
                    t3 = tmp_pool.tile([64, SQ], f32, tag="ropetmp3")
                    t4 = tmp_pool.tile([64, SQ], f32, tag="ropetmp4")
                    nc.vector.tensor_mul(t3[:], te, sin_sb[:])
                    nc.vector.tensor_mul(t4[:], to, cos_sb[:])
                    nc.vector.tensor_add(dst[64:128, dcs], t3[:], t4[:])

            # ---- K projection + RoPE -> k_in, AllGather ----
            with (
                tc.tile_pool(name="projps", bufs=8, space="PSUM") as projps,
                tc.tile_pool(name="ropetmp", bufs=3) as ropep,
            ):
                # ---- V projection (natural layout) -> v_in, AllGather ----
                for ncol in range(4):
                    vps = [projps.tile([128, 512], f32, tag="projps", name="projtile")
                           for _ in range(4)]
                    for d in range(DCH):
                        wvt = wpool.tile([128, 512], bf, tag="wst")
                        nc.sync.dma_start(
                            out=wvt[:],
                            in_=wv[d * 128:(d + 1) * 128, ncol * 512:(ncol + 1) * 512],
                        )
                        for rr in range(4):
                            nc.tensor.matmul(
                                vps[rr][:],
                                lhsT=xts[:, d * ROWS + rr * 128:d * ROWS + (rr + 1) * 128],
                                rhs=wvt[:],
                                start=(d == 0), stop=(d == DCH - 1),
                            )
                    for rr in range(4):
                        vsb = sbout.tile([128, 512], bf, tag="vsb")
                        nc.scalar.copy(vsb[:], vps[rr][:])
                        nc.scalar.dma_start(
                            out=v_in[rr * 128:(rr + 1) * 128, ncol * 512:(ncol + 1) * 512],
                            in_=vsb[:],
                        )
                nc.gpsimd.collective_compute(
                    "AllGather",
                    mybir.AluOpType.bypass,
                    replica_groups=[list(range(NCORES))],
                    ins=[v_in.opt()],
                    outs=[v_out.opt()],
                )

                # ---- K projection + RoPE -> k_in, AllGather ----
                for hg in range(4):          # head groups of 4
                    kps = [projps.tile([128, ROWS], f32, tag="projps", name="projtile")
                           for _ in range(4)]
                    for d in range(DCH):
                        wkt = wpool.tile([128, 512], bf, tag="wst")
                        nc.sync.dma_start(
                            out=wkt[:],
                            in_=wk[d * 128:(d + 1) * 128, hg * 512:(hg + 1) * 512],
                        )
                        for hh in range(4):
                            nc.tensor.matmul(
                                kps[hh][:],
                                lhsT=wkt[:, hh * 128:(hh + 1) * 128],
                                rhs=xts[:, d * ROWS:(d + 1) * ROWS],
                                start=(d == 0), stop=(d == DCH - 1),
                            )
                    for hh in range(4):
                        h = hg * 4 + hh
                        ksb = sbout.tile([128, ROWS], bf, tag="ksb")
                        rope(ksb, 0, kps[hh], ropep)
                        nc.scalar.dma_start(
                            out=k_in[h * 128:(h + 1) * 128, :], in_=ksb[:]
                        )
                    nc.gpsimd.collective_compute(
                        "AllGather",
                        mybir.AluOpType.bypass,
                        replica_groups=[list(range(NCORES))],
                        ins=[k_in[hg * 512:(hg + 1) * 512, :].opt()],
                        outs=[k_outs[hg].opt()],
                    )

                # ---- Q projection + RoPE (stays in SBUF) ----
                for hg in range(4):
                    qps = [projps.tile([128, ROWS], f32, tag="projps", name="projtile")
                           for _ in range(4)]
                    for d in range(DCH):
                        wqt = wpool.tile([128, 512], bf, tag="wst")
                        nc.sync.dma_start(
                            out=wqt[:],
                            in_=wq[d * 128:(d + 1) * 128, hg * 512:(hg + 1) * 512],
                        )
                        for hh in range(4):
                            nc.tensor.matmul(
                                qps[hh][:],
                                lhsT=wqt[:, hh * 128:(hh + 1) * 128],
                                rhs=xts[:, d * ROWS:(d + 1) * ROWS],
                                start=(d == 0), stop=(d == DCH - 1),
                            )
                    for hh in range(4):
                        h = hg * 4 + hh
                        rope(qsb, h * ROWS, qps[hh], ropep)

            # rank-major views of the gathered K^T and V for one-DMA-per-head
            # loads: K rows = rk*2048 + h*128 + d, V rows = cc*128 + p.
            k_views = [
                k_outs[i][:].rearrange(
                    "(rk h d) c -> h d rk c", rk=NCORES, h=4, d=128
                )
                for i in range(4)
            ]
            v_view = v_out[:].rearrange(
                "(cc p) (h t) -> h p cc t", cc=4 * NCORES, p=128, h=H, t=128
            )

            for kcb in range(16):
                nc.gpsimd.dma_start(
                    out=mask_sb[:, kcb * SQ:(kcb + 1) * SQ],
                    in_=maskT[kcb * 128:(kcb + 1) * 128, :],
                )

            # ---- Attention per head ----
            with (
                tc.tile_pool(name="ktiles", bufs=4) as kpool,
                tc.tile_pool(name="vtiles", bufs=4) as vpool,
                tc.tile_pool(name="scps", bufs=2, space="PSUM") as scps,
                tc.tile_pool(name="attps", bufs=4, space="PSUM") as attps,
                tc.tile_pool(name="smallps", bufs=1, space="PSUM") as smallps,
                tc.tile_pool(name="extiles", bufs=8) as expool,
                tc.tile_pool(name="tmp", bufs=3) as tmpp,
            ):
                for h in range(H):
                    kta = kpool.tile([128, NCORES * ROWS], bf, tag="kt")
                    nc.gpsimd.dma_start(out=kta[:], in_=k_views[h // 4][h % 4])
                    vta = vpool.tile([128, NCORES * ROWS], bf, tag="vt")
                    nc.gpsimd.dma_start(out=vta[:], in_=v_view[h])
                    att_ps = [
                        attps.tile([128, SQ], f32, tag="attps", name="atttile")
                        for _ in range(B)
                    ]
                    den = smallps.tile([1, ROWS], f32, tag="den")
                    for kc in range(16):
                        rk, koff = kc // 2, (kc % 2) * 128
                        sc = scps.tile([128, ROWS], f32, tag="scps")
                        for b2 in range(B):
                            nc.tensor.matmul(
                                sc[:, b2 * SQ:(b2 + 1) * SQ],
                                lhsT=kta[:, rk * ROWS + b2 * SQ + koff:
                                         rk * ROWS + b2 * SQ + koff + 128],
                                rhs=qsb[:, h * ROWS + b2 * SQ:h * ROWS + (b2 + 1) * SQ],
                                start=True, stop=True,
                            )
                        ex = expool.tile([128, ROWS], bf, tag="ex")
                        tmp = tmpp.tile([128, ROWS], f32, tag="tmp")
                        for b2 in range(B):
                            cs = slice(b2 * SQ, (b2 + 1) * SQ)
                            nc.vector.tensor_add(
                                tmp[:, cs], sc[:, cs],
                                mask_sb[:, kc * SQ:(kc + 1) * SQ],
                            )
                        nc.scalar.activation(ex[:], tmp[:], Exp, scale=SCALE)
                        # denominator: accumulate column sums of exp on the PE
                        nc.tensor.matmul(
                            den[:], lhsT=ones_sq[:, 0:1], rhs=ex[:],
                            start=(kc == 0), stop=(kc == 15),
                        )
                        for b2 in range(B):
                            ccb = rk * 4 + b2 * 2 + koff // 128
                            nc.tensor.matmul(
                                att_ps[b2][:],
                                lhsT=vta[:, ccb * 128:(ccb + 1) * 128],
                                rhs=ex[:, b2 * SQ:(b2 + 1) * SQ],
                                start=(kc == 0), stop=(kc == 15),
                            )
                    dsb = tmpp.tile([1, ROWS], bf, tag="dsb")
                    nc.scalar.copy(dsb[:], den[:])
                    rb = smallps.tile([128, ROWS], f32, tag="rb")
                    nc.tensor.matmul(
                        rb[:], lhsT=ones_sq[0:1, :], rhs=dsb[:],
                        start=True, stop=True,
                    )
                    rbs = tmpp.tile([128, ROWS], f32, tag="rbs")
                    rscr = tmpp.tile([128, ROWS], f32, tag="rscr")
                    nc.vector.reciprocal_approx_accurate(rbs[:], rb[:], rscr[:])
                    for b2 in range(B):
                        nc.vector.tensor_mul(
                            attn[:, h * ROWS + b2 * SQ:h * ROWS + (b2 + 1) * SQ],
                            att_ps[b2][:],
                            rbs[:, b2 * SQ:(b2 + 1) * SQ],
                        )

            # ---- Output projection ----
            with tc.tile_pool(name="ops", bufs=8, space="PSUM") as opsp:
                for nn in range(4):
                    ops = [opsp.tile([128, 512], f32, tag="ops", name="opstile")
                           for _ in range(4)]
                    for h in range(H):
                        wot = wpool.tile([128, 512], bf, tag="wst")
                        nc.sync.dma_start(
                            out=wot[:],
                            in_=wo[h * 128:(h + 1) * 128, nn * 512:(nn + 1) * 512],
                        )
                        for qt in range(4):
                            nc.tensor.matmul(
                                ops[qt][:],
                                lhsT=attn[:, h * ROWS + qt * 128:h * ROWS + (qt + 1) * 128],
                                rhs=wot[:],
                                start=(h == 0), stop=(h == H - 1),
                            )
                    for qt in range(4):
                        osb = sbout.tile([128, 512], f32, tag="osb")
                        nc.scalar.copy(osb[:], ops[qt][:])
                        nc.sync.dma_start(
                            out=out[qt * 128:(qt + 1) * 128, nn * 512:(nn + 1) * 512],
                            in_=osb[:],
                        )

    nc.compile()
    return nc


def _get_graph():
    global _GRAPH
    if _GRAPH is None:
        _GRAPH = _build_graph()
    return _GRAPH


_PERM = np.concatenate(
    [h * HD + np.concatenate([np.arange(0, HD, 2), np.arange(1, HD, 2)])
     for h in range(H)]
)


def kernel(x, Wq, Wk, Wv, Wo, freqs_cos, freqs_sin, mask):
    global _LAST_EXEC_NS, _LAST_RES
    from concourse.bass_utils import run_bass_kernel_spmd

    nc = _get_graph()

    x = np.asarray(x, np.float32)
    wq_p = np.ascontiguousarray(np.asarray(Wq, np.float32)[:, _PERM]).astype(BF16)
    wk_p = np.ascontiguousarray(np.asarray(Wk, np.float32)[:, _PERM]).astype(BF16)
    wv_b = np.ascontiguousarray(np.asarray(Wv, np.float32)).astype(BF16)
    wo_b = np.ascontiguousarray(np.asarray(Wo, np.float32)).astype(BF16)
    cosf = np.asarray(freqs_cos, np.float32)
    sinf = np.asarray(freqs_sin, np.float32)
    maskf = np.asarray(mask, np.float32)[0, 0]      # [S, S] (q, k)
    ones_b = np.ones((128, 128), BF16)

    in_maps = []
    for c in range(NCORES):
        rows = slice(c * SQ, (c + 1) * SQ)
        x_c = x[:, rows, :].reshape(ROWS, D)
        in_maps.append({
            "xT": np.ascontiguousarray(x_c.T).astype(BF16),
            "wq": wq_p, "wk": wk_p, "wv": wv_b, "wo": wo_b,
            "cosT": np.ascontiguousarray(cosf[rows].T),
            "sinT": np.ascontiguousarray(sinf[rows].T),
            "maskT": np.ascontiguousarray(maskf[rows].T * float(np.sqrt(HD))),
            "ones": ones_b,
        })

    res = run_bass_kernel_spmd(
        nc, in_maps, core_ids=list(range(NCORES)), trace=_TRACE,
    )
    _LAST_EXEC_NS = res.exec_time_ns
    _LAST_RES = res

    outp = np.empty((B, S, D), np.float32)
    for c in range(NCORES):
        o = res.results[c]["out"]
        for b in range(B):
            outp[b, c * SQ:(c + 1) * SQ, :] = o[b * SQ:(b + 1) * SQ, :]
    return outp



# revision 3
# speedup vs baseline: 1.3598x; 1.3598x over previous
"""Distributed Bass kernel for causal multi-head attention with RoPE.

Problem: B=2, S=2048, D=2048, H=16, HD=128 (nn_Attention_85315230368481).

Sharding: tensor-parallel over heads. Core c owns heads (2c, 2c+1); x is
replicated. Each core projects Q/K/V for its 2 heads over the full
sequence, applies RoPE, computes causal attention, and multiplies by its
row-slice of Wo, producing a partial [B*S, D] output. The host sums the
8 partials (the output is sum-sharded over head groups) — no device
collectives at all.

Performance structure:
 - Causal tile skipping: per 512-query chunk qc only key tiles
   kt <= 4*qc+3 are computed (~37% of score/attnV/exp work skipped).
   The causal mask inside the diagonal band is applied by a single
   gpsimd.affine_select per tile (no mask tensor, no DVE adds).
 - Softmax denominator via an all-ones [128,128] stationary matmul that
   also broadcasts the row sums across all partitions for free; the
   divide is folded into the PSUM-evacuation scalar_tensor_tensor.
 - Everything bf16 into the PE (fp32 PSUM accumulate); RoPE runs in
   bf16 split across DVE and GpSimd.
 - Batches processed one at a time so x^T stays SBUF-resident per batch;
   batch 1's x/weight DMAs overlap batch 0's attention.
"""

import sys

import ml_dtypes
import numpy as np

if "/opt/trn_rl_repo" not in sys.path:
    sys.path.insert(0, "/opt/trn_rl_repo")

B, S, D, H = 2, 2048, 2048, 16
HD = D // H            # 128
NCORES = 8
HP = H // NCORES       # 2 heads per core
NCH = D // 128         # 16 contraction chunks
NP = S // 512          # 4 row pieces per batch (projection)
NQC = S // 512         # 4 query chunks per batch (attention)
NKT = S // 128         # 16 key tiles per batch
SCALE = 1.0 / float(np.sqrt(HD))
BF16 = ml_dtypes.bfloat16

_GRAPH = None
_TRACE = False
_LAST_EXEC_NS = None
_LAST_RES = None


def _build_graph():
    import concourse.mybir as mybir
    from concourse import bacc, tile

    f32 = mybir.dt.float32
    bf = mybir.dt.bfloat16
    Exp = mybir.ActivationFunctionType.Exp

    nc = bacc.Bacc("TRN2", target_bir_lowering=False, num_devices=NCORES)

    xT = nc.declare_dram_parameter("xT", [D, B * S], bf, isOutput=False)
    wq = nc.declare_dram_parameter("wq", [D, HP * HD], bf, isOutput=False)
    wk = nc.declare_dram_parameter("wk", [D, HP * HD], bf, isOutput=False)
    wv = nc.declare_dram_parameter("wv", [D, HP * HD], bf, isOutput=False)
    wo = nc.declare_dram_parameter("wo", [HP * HD, D], bf, isOutput=False)
    cosT = nc.declare_dram_parameter("cosT", [64, S], bf, isOutput=False)
    sinT = nc.declare_dram_parameter("sinT", [64, S], bf, isOutput=False)
    onesd = nc.declare_dram_parameter("ones", [128, 128], bf, isOutput=False)
    out = nc.declare_dram_parameter("out", [B * S, D], bf, isOutput=True)

    with nc.allow_low_precision(reason="bf16 matmul inputs; fp32 accumulate"), \
         tile.TileContext(nc) as tc:
        with (
            tc.tile_pool(name="const", bufs=1) as constp,
            tc.tile_pool(name="xpool", bufs=1) as xpool,
            tc.tile_pool(name="kqv", bufs=2) as kqvp,
            tc.tile_pool(name="wstream", bufs=2) as wpool,
            tc.tile_pool(name="teto", bufs=4) as tetop,
            tc.tile_pool(name="ropetmp", bufs=4) as rtp,
            tc.tile_pool(name="extiles", bufs=6) as expool,
            tc.tile_pool(name="rpool", bufs=2) as rpool,
            tc.tile_pool(name="ostage", bufs=2) as ostag,
            tc.tile_pool(name="projps", bufs=2, space="PSUM") as projps,
            tc.tile_pool(name="scps", bufs=2, space="PSUM") as scps,
            tc.tile_pool(name="attps", bufs=2, space="PSUM") as attps,
            tc.tile_pool(name="denps", bufs=2, space="PSUM") as denps,
        ):
            cos_sb = constp.tile([64, S], bf)
            sin_sb = constp.tile([64, S], bf)
            ones_sb = constp.tile([128, 128], bf)
            nc.sync.dma_start(out=cos_sb[:], in_=cosT[:, :])
            nc.sync.dma_start(out=sin_sb[:], in_=sinT[:, :])
            nc.sync.dma_start(out=ones_sb[:], in_=onesd[:, :])

            for b in range(B):
                # ---- load x^T for this batch ----
                xts = xpool.tile([128, NCH * S], bf, tag="xts", name="xts")
                for c in range(NCH):
                    nc.sync.dma_start(
                        out=xts[:, c * S:(c + 1) * S],
                        in_=xT[c * 128:(c + 1) * 128, b * S:(b + 1) * S],
                    )

                ktsb = kqvp.tile([128, HP * S], bf, tag="kt", name="ktsb")
                qasb = kqvp.tile([128, HP * S], bf, tag="qa", name="qasb")
                vsb = kqvp.tile([128, NKT * HP * HD], bf, tag="v", name="vsb")

                # ---- K / Q projections + RoPE ----
                for wdram, dst in ((wk, ktsb), (wq, qasb)):
                    wsb = wpool.tile([128, NCH * 256], bf, tag="w", name="wsb")
                    for c in range(NCH):
                        nc.sync.dma_start(
                            out=wsb[:, c * 256:(c + 1) * 256],
                            in_=wdram[c * 128:(c + 1) * 128, :],
                        )
                    for piece in range(NP):
                        for ht in range(HP):
                            pp = projps.tile([128, 512], f32, tag="pp",
                                             name="pptile")
                            for c in range(NCH):
                                nc.tensor.matmul(
                                    pp[:],
                                    lhsT=wsb[:, c * 256 + ht * 128:
                                             c * 256 + (ht + 1) * 128],
                                    rhs=xts[:, c * S + piece * 512:
                                            c * S + piece * 512 + 512],
                                    start=(c == 0), stop=(c == NCH - 1),
                                )
                            te = tetop.tile([64, 512], bf, tag="te",
                                            name="te")
                            to = tetop.tile([64, 512], bf, tag="to",
                                            name="to")
                            nc.scalar.copy(te[:], pp[0:64, :])
                            nc.scalar.copy(to[:], pp[64:128, :])
                            pc = slice(piece * 512, piece * 512 + 512)
                            dc = slice(ht * S + piece * 512,
                                       ht * S + piece * 512 + 512)
                            te = te[:]
                            to = to[:]
                            t1 = rtp.tile([64, 512], bf, tag="t1", name="t1")
                            t2 = rtp.tile([64, 512], bf, tag="t2", name="t2")
                            nc.vector.tensor_mul(t1[:], te, cos_sb[:, pc])
                            nc.vector.tensor_mul(t2[:], to, sin_sb[:, pc])
                            nc.vector.tensor_sub(dst[0:64, dc], t1[:], t2[:])
                            t3 = rtp.tile([64, 512], bf, tag="t3", name="t3")
                            t4 = rtp.tile([64, 512], bf, tag="t4", name="t4")
                            nc.gpsimd.tensor_mul(t3[:], te, sin_sb[:, pc])
                            nc.gpsimd.tensor_mul(t4[:], to, cos_sb[:, pc])
                            nc.gpsimd.tensor_add(dst[64:128, dc], t3[:], t4[:])

                # ---- V projection (natural layout) ----
                wsb = wpool.tile([128, NCH * 256], bf, tag="w", name="wsb")
                for c in range(NCH):
                    nc.sync.dma_start(
                        out=wsb[:, c * 256:(c + 1) * 256],
                        in_=wv[c * 128:(c + 1) * 128, :],
                    )
                for rt in range(NKT):
                    vp = denps.tile([128, 512], f32, tag="den", name="dentile")
                    for c in range(NCH):
                        nc.tensor.matmul(
                            vp[:, 0:256],
                            lhsT=xts[:, c * S + rt * 128:c * S + (rt + 1) * 128],
                            rhs=wsb[:, c * 256:(c + 1) * 256],
                            start=(c == 0), stop=(c == NCH - 1),
                        )
                    nc.scalar.copy(vsb[:, rt * 256:(rt + 1) * 256],
                                   vp[:, 0:256])

                # ---- attention per head ----
                for h in range(HP):
                    for qc in range(NQC):
                        nkt = 4 * qc + 4          # live key tiles
                        att = attps.tile([128, 512], f32, tag="att",
                                         name="atttile")
                        den = denps.tile([128, 512], f32, tag="den",
                                         name="dentile")
                        for kt in range(nkt):
                            sc = scps.tile([128, 512], f32, tag="sc",
                                           name="sctile")
                            nc.tensor.matmul(
                                sc[:],
                                lhsT=ktsb[:, h * S + kt * 128:
                                          h * S + (kt + 1) * 128],
                                rhs=qasb[:, h * S + qc * 512:
                                         h * S + qc * 512 + 512],
                                start=True, stop=True,
                            )
                            ex = expool.tile([128, 512], bf, tag="ex",
                                             name="ex")
                            nc.scalar.activation(ex[:], sc[:], Exp,
                                                 scale=SCALE)
                            if kt >= 4 * qc:
                                # diagonal band: zero where key > query
                                nc.gpsimd.affine_select(
                                    out=ex[:], in_=ex[:],
                                    pattern=[[1, 512]],
                                    compare_op=mybir.AluOpType.is_ge,
                                    fill=0.0,
                                    base=qc * 512 - kt * 128,
                                    channel_multiplier=-1,
                                )
                            nc.tensor.matmul(
                                den[:], lhsT=ones_sb[:, :], rhs=ex[:],
                                start=(kt == 0), stop=(kt == nkt - 1),
                            )
                            nc.tensor.matmul(
                                att[:],
                                lhsT=vsb[:, kt * 256 + h * 128:
                                         kt * 256 + (h + 1) * 128],
                                rhs=ex[:],
                                start=(kt == 0), stop=(kt == nkt - 1),
                            )
                        rsb = rpool.tile([128, 512], f32, tag="rc", name="rsb")
                        rscr = rpool.tile([128, 512], f32, tag="rs",
                                          name="rscr")
                        nc.vector.reciprocal_approx_accurate(rsb[:], den[:],
                                                             rscr[:])
                        nc.vector.scalar_tensor_tensor(
                            out=qasb[:, h * S + qc * 512:
                                     h * S + qc * 512 + 512],
                            in0=att[:], scalar=1.0, in1=rsb[:],
                            op0=mybir.AluOpType.mult,
                            op1=mybir.AluOpType.mult,
                        )

                # ---- output projection (partial over this core's heads) ----
                wosb = wpool.tile([128, HP * D], bf, tag="wo", name="wosb")
                for ht in range(HP):
                    nc.sync.dma_start(
                        out=wosb[:, ht * D:(ht + 1) * D],
                        in_=wo[ht * 128:(ht + 1) * 128, :],
                    )
                for qt in range(NKT):
                    osb = ostag.tile([128, D], bf, tag="o", name="osb")
                    for ncol in range(4):
                        op = projps.tile([128, 512], f32, tag="pp",
                                         name="pptile")
                        for ht in range(HP):
                            nc.tensor.matmul(
                                op[:],
                                lhsT=qasb[:, ht * S + qt * 128:
                                          ht * S + (qt + 1) * 128],
                                rhs=wosb[:, ht * D + ncol * 512:
                                         ht * D + ncol * 512 + 512],
                                start=(ht == 0), stop=(ht == HP - 1),
                            )
                        nc.scalar.copy(osb[:, ncol * 512:(ncol + 1) * 512],
                                       op[:])
                    nc.sync.dma_start(
                        out=out[b * S + qt * 128:b * S + (qt + 1) * 128, :],
                        in_=osb[:],
                    )

    nc.compile()
    return nc


def _get_graph():
    global _GRAPH
    if _GRAPH is None:
        _GRAPH = _build_graph()
    return _GRAPH


# per-head column permutation: even dims then odd dims (RoPE partition trick)
_EVOD = np.concatenate([np.arange(0, HD, 2), np.arange(1, HD, 2)])


def kernel(x, Wq, Wk, Wv, Wo, freqs_cos, freqs_sin, mask):
    global _LAST_EXEC_NS, _LAST_RES
    from concourse.bass_utils import run_bass_kernel_spmd

    nc = _get_graph()

    x = np.asarray(x, np.float32).reshape(B * S, D)
    xTb = np.ascontiguousarray(x.T).astype(BF16)
    cos_b = np.ascontiguousarray(np.asarray(freqs_cos, np.float32).T).astype(BF16)
    sin_b = np.ascontiguousarray(np.asarray(freqs_sin, np.float32).T).astype(BF16)
    ones_b = np.ones((128, 128), BF16)
    Wq = np.asarray(Wq, np.float32)
    Wk = np.asarray(Wk, np.float32)
    Wv = np.asarray(Wv, np.float32)
    Wo = np.asarray(Wo, np.float32)

    in_maps = []
    for c in range(NCORES):
        pcols = np.concatenate([(2 * c + j) * HD + _EVOD for j in range(HP)])
        ncols = slice(2 * c * HD, (2 * c + HP) * HD)
        in_maps.append({
            "xT": xTb,
            "wq": np.ascontiguousarray(Wq[:, pcols]).astype(BF16),
            "wk": np.ascontiguousarray(Wk[:, pcols]).astype(BF16),
            "wv": np.ascontiguousarray(Wv[:, ncols]).astype(BF16),
            "wo": np.ascontiguousarray(Wo[ncols, :]).astype(BF16),
            "cosT": cos_b,
            "sinT": sin_b,
            "ones": ones_b,
        })

    res = run_bass_kernel_spmd(
        nc, in_maps, core_ids=list(range(NCORES)), trace=_TRACE,
    )
    _LAST_EXEC_NS = res.exec_time_ns
    _LAST_RES = res

    acc = np.zeros((B * S, D), np.float32)
    for c in range(NCORES):
        acc += res.results[c]["out"].astype(np.float32)
    return acc.reshape(B, S, D)


# revision 8
# speedup vs baseline: 1.5380x; 1.1311x over previous
"""Distributed Bass kernel for causal multi-head attention with RoPE.

Problem: B=2, S=2048, D=2048, H=16, HD=128 (nn_Attention_85315230368481).

Sharding: tensor-parallel over heads. Core c owns heads (2c, 2c+1); x is
replicated. Each core projects Q/K/V for its 2 heads over the full
sequence, applies RoPE, computes causal attention, and multiplies by its
row-slice of Wo, producing a partial [B*S, D] output. The host sums the
8 partials (the output is sum-sharded over head groups) — no device
collectives at all.

Performance structure:
 - Causal tile skipping: per 512-query chunk qc only key tiles
   kt <= 4*qc+3 are computed (~37% of score/attnV/exp work skipped).
   The causal mask inside the diagonal band is applied by a single
   gpsimd.affine_select per tile (no mask tensor, no DVE adds).
 - Softmax denominator via an all-ones [128,128] stationary matmul that
   also broadcasts the row sums across all partitions for free; the
   divide is folded into the PSUM-evacuation scalar_tensor_tensor.
 - Everything bf16 into the PE (fp32 PSUM accumulate); RoPE runs in
   bf16 split across DVE and GpSimd.
 - Batches processed one at a time so x^T stays SBUF-resident per batch;
   batch 1's x/weight DMAs overlap batch 0's attention.
"""

import sys

import ml_dtypes
import numpy as np

if "/opt/trn_rl_repo" not in sys.path:
    sys.path.insert(0, "/opt/trn_rl_repo")

B, S, D, H = 2, 2048, 2048, 16
HD = D // H            # 128
NCORES = 8
HP = H // NCORES       # 2 heads per core
NCH = D // 128         # 16 contraction chunks
NP = S // 512          # 4 row pieces per batch (projection)
NQC = S // 512         # 4 query chunks per batch (attention)
NKT = S // 128         # 16 key tiles per batch
SCALE = 1.0 / float(np.sqrt(HD))
BF16 = ml_dtypes.bfloat16

_GRAPH = None
_TRACE = False
_LAST_EXEC_NS = None
_LAST_RES = None


def _build_graph():
    import concourse.mybir as mybir
    from concourse import bacc, tile

    f32 = mybir.dt.float32
    bf = mybir.dt.bfloat16
    Exp = mybir.ActivationFunctionType.Exp

    nc = bacc.Bacc("TRN2", target_bir_lowering=False, num_devices=NCORES)

    xT = nc.declare_dram_parameter("xT", [D, B * S], bf, isOutput=False)
    wq = nc.declare_dram_parameter("wq", [D, HP * HD], bf, isOutput=False)
    wk = nc.declare_dram_parameter("wk", [D, HP * HD], bf, isOutput=False)
    wv = nc.declare_dram_parameter("wv", [D, HP * HD], bf, isOutput=False)
    wo = nc.declare_dram_parameter("wo", [HP * HD, D], bf, isOutput=False)
    cosT = nc.declare_dram_parameter("cosT", [64, S], bf, isOutput=False)
    sinT = nc.declare_dram_parameter("sinT", [64, S], bf, isOutput=False)
    onesd = nc.declare_dram_parameter("ones", [128, 128], bf, isOutput=False)
    out = nc.declare_dram_parameter("out", [B * S, D], bf, isOutput=True)

    with nc.allow_low_precision(reason="bf16 matmul inputs; fp32 accumulate"), \
         tile.TileContext(nc) as tc:
        with (
            tc.tile_pool(name="const", bufs=1) as constp,
            tc.tile_pool(name="xpool", bufs=1) as xpool,
            tc.tile_pool(name="kqv", bufs=2) as kqvp,
            tc.tile_pool(name="wstream", bufs=2) as wpool,
            tc.tile_pool(name="teto", bufs=4) as tetop,
            tc.tile_pool(name="ropetmp", bufs=4) as rtp,
            tc.tile_pool(name="extiles", bufs=6) as expool,
            tc.tile_pool(name="rpool", bufs=2) as rpool,
            tc.tile_pool(name="ostage", bufs=2) as ostag,
            tc.tile_pool(name="projps", bufs=2, space="PSUM") as projps,
            tc.tile_pool(name="scps", bufs=2, space="PSUM") as scps,
            tc.tile_pool(name="attps", bufs=2, space="PSUM") as attps,
            tc.tile_pool(name="denps", bufs=2, space="PSUM") as denps,
        ):
            # cos/sin duplicated on partitions 0-63 and 64-127 so RoPE's
            # tensor_tensor inputs can share a start partition in both halves
            cos_sb = constp.tile([128, S], bf)
            sin_sb = constp.tile([128, S], bf)
            ones_sb = constp.tile([128, 128], bf)
            nc.sync.dma_start(out=cos_sb[0:64, :], in_=cosT[:, :])
            nc.sync.dma_start(out=cos_sb[64:128, :], in_=cosT[:, :])
            nc.sync.dma_start(out=sin_sb[0:64, :], in_=sinT[:, :])
            nc.sync.dma_start(out=sin_sb[64:128, :], in_=sinT[:, :])
            nc.sync.dma_start(out=ones_sb[:], in_=onesd[:, :])

            for b in range(B):
                # ---- load x^T for this batch ----
                xts = xpool.tile([128, NCH * S], bf, tag="xts", name="xts")
                for c in range(NCH):
                    nc.sync.dma_start(
                        out=xts[:, c * S:(c + 1) * S],
                        in_=xT[c * 128:(c + 1) * 128, b * S:(b + 1) * S],
                    )

                ktsb = kqvp.tile([128, HP * S], bf, tag="kt", name="ktsb")
                qasb = kqvp.tile([128, HP * S], bf, tag="qa", name="qasb")
                vsb = kqvp.tile([128, NKT * HP * HD], bf, tag="v", name="vsb")

                # ---- K / Q projections + RoPE ----
                for wdram, dst in ((wk, ktsb), (wq, qasb)):
                    wsb = wpool.tile([128, NCH * 256], bf, tag="w", name="wsb")
                    for c in range(NCH):
                        nc.sync.dma_start(
                            out=wsb[:, c * 256:(c + 1) * 256],
                            in_=wdram[c * 128:(c + 1) * 128, :],
                        )
                    for piece in range(NP):
                        for ht in range(HP):
                            pp = projps.tile([128, 512], f32, tag="pp",
                                             name="pptile")
                            for c in range(NCH):
                                nc.tensor.matmul(
                                    pp[:],
                                    lhsT=wsb[:, c * 256 + ht * 128:
                                             c * 256 + (ht + 1) * 128],
                                    rhs=xts[:, c * S + piece * 512:
                                            c * S + piece * 512 + 512],
                                    start=(c == 0), stop=(c == NCH - 1),
                                )
                            teto = tetop.tile([128, 512], bf, tag="teto",
                                              name="teto")
                            nc.scalar.copy(teto[:], pp[:])
                            pc = slice(piece * 512, piece * 512 + 512)
                            pc2 = slice(piece * 512, piece * 512 + 512)
                            dc = slice(ht * S + piece * 512,
                                       ht * S + piece * 512 + 512)
                            te = teto[0:64, :]
                            to = teto[64:128, :]
                            cos_lo = cos_sb[0:64, pc]
                            cos_hi = cos_sb[64:128, pc2]
                            sin_lo = sin_sb[0:64, pc]
                            sin_hi = sin_sb[64:128, pc2]
                            t1 = rtp.tile([64, 512], bf, tag="t1", name="t1")
                            t2 = rtp.tile([64, 512], bf, tag="t2", name="t2")
                            nc.vector.tensor_mul(t1[:], te, cos_lo)
                            nc.vector.tensor_mul(t2[:], to, sin_hi)
                            nc.vector.tensor_sub(dst[0:64, dc], t1[:], t2[:])
                            t3 = rtp.tile([64, 512], bf, tag="t3", name="t3")
                            t4 = rtp.tile([64, 512], bf, tag="t4", name="t4")
                            nc.gpsimd.tensor_mul(t3[:], te, sin_lo)
                            nc.gpsimd.tensor_mul(t4[:], to, cos_hi)
                            nc.gpsimd.tensor_add(dst[64:128, dc], t3[:], t4[:])

                # ---- V projection (natural layout) ----
                wsb = wpool.tile([128, NCH * 256], bf, tag="w", name="wsb")
                for c in range(NCH):
                    nc.sync.dma_start(
                        out=wsb[:, c * 256:(c + 1) * 256],
                        in_=wv[c * 128:(c + 1) * 128, :],
                    )
                for rt in range(NKT):
                    vp = denps.tile([128, 512], f32, tag="den", name="dentile")
                    for c in range(NCH):
                        nc.tensor.matmul(
                            vp[:, 0:256],
                            lhsT=xts[:, c * S + rt * 128:c * S + (rt + 1) * 128],
                            rhs=wsb[:, c * 256:(c + 1) * 256],
                            start=(c == 0), stop=(c == NCH - 1),
                        )
                    nc.vector.tensor_copy(vsb[:, rt * 256:(rt + 1) * 256],
                                          vp[:, 0:256])

                # ---- attention per head ----
                for h in range(HP):
                    for qc in range(NQC):
                        nkt = 4 * qc + 4          # live key tiles
                        att = attps.tile([128, 512], f32, tag="att",
                                         name="atttile")
                        den = denps.tile([128, 512], f32, tag="den",
                                         name="dentile")
                        for kt in range(nkt):
                            sc = scps.tile([128, 512], f32, tag="sc",
                                           name="sctile")
                            nc.tensor.matmul(
                                sc[:],
                                lhsT=ktsb[:, h * S + kt * 128:
                                          h * S + (kt + 1) * 128],
                                rhs=qasb[:, h * S + qc * 512:
                                         h * S + qc * 512 + 512],
                                start=True, stop=True,
                            )
                            ex = expool.tile([128, 512], bf, tag="ex",
                                             name="ex")
                            nc.scalar.activation(ex[:], sc[:], Exp,
                                                 scale=SCALE)
                            if kt >= 4 * qc:
                                # diagonal band: zero where key > query
                                nc.gpsimd.affine_select(
                                    out=ex[:], in_=ex[:],
                                    pattern=[[1, 512]],
                                    compare_op=mybir.AluOpType.is_ge,
                                    fill=0.0,
                                    base=qc * 512 - kt * 128,
                                    channel_multiplier=-1,
                                )
                            nc.tensor.matmul(
                                den[:], lhsT=ones_sb[:, :], rhs=ex[:],
                                start=(kt == 0), stop=(kt == nkt - 1),
                            )
                            nc.tensor.matmul(
                                att[:],
                                lhsT=vsb[:, kt * 256 + h * 128:
                                         kt * 256 + (h + 1) * 128],
                                rhs=ex[:],
                                start=(kt == 0), stop=(kt == nkt - 1),
                            )
                        rsb = rpool.tile([128, 512], f32, tag="rc", name="rsb")
                        nc.vector.reciprocal_approx_fast(rsb[:], den[:])
                        nc.vector.scalar_tensor_tensor(
                            out=qasb[:, h * S + qc * 512:
                                     h * S + qc * 512 + 512],
                            in0=att[:], scalar=1.0, in1=rsb[:],
                            op0=mybir.AluOpType.mult,
                            op1=mybir.AluOpType.mult,
                        )

                # ---- output projection (partial over this core's heads) ----
                wosb = wpool.tile([128, HP * D], bf, tag="wo", name="wosb")
                for ht in range(HP):
                    nc.sync.dma_start(
                        out=wosb[:, ht * D:(ht + 1) * D],
                        in_=wo[ht * 128:(ht + 1) * 128, :],
                    )
                for qt in range(NKT):
                    osb = ostag.tile([128, D], bf, tag="o", name="osb")
                    for ncol in range(4):
                        op = projps.tile([128, 512], f32, tag="pp",
                                         name="pptile")
                        for ht in range(HP):
                            nc.tensor.matmul(
                                op[:],
                                lhsT=qasb[:, ht * S + qt * 128:
                                          ht * S + (qt + 1) * 128],
                                rhs=wosb[:, ht * D + ncol * 512:
                                         ht * D + ncol * 512 + 512],
                                start=(ht == 0), stop=(ht == HP - 1),
                            )
                        oslice = osb[:, ncol * 512:(ncol + 1) * 512]
                        if ncol % 2 == 0:
                            nc.scalar.copy(oslice, op[:])
                        else:
                            nc.vector.tensor_copy(oslice, op[:])
                    nc.sync.dma_start(
                        out=out[b * S + qt * 128:b * S + (qt + 1) * 128, :],
                        in_=osb[:],
                    )

    nc.compile()
    return nc


def _get_graph():
    global _GRAPH
    if _GRAPH is None:
        _GRAPH = _build_graph()
    return _GRAPH


# per-head column permutation: even dims then odd dims (RoPE partition trick)
_EVOD = np.concatenate([np.arange(0, HD, 2), np.arange(1, HD, 2)])


def kernel(x, Wq, Wk, Wv, Wo, freqs_cos, freqs_sin, mask):
    global _LAST_EXEC_NS, _LAST_RES
    from concourse.bass_utils import run_bass_kernel_spmd

    nc = _get_graph()

    x = np.asarray(x, np.float32).reshape(B * S, D)
    xTb = np.ascontiguousarray(x.T).astype(BF16)
    cos_b = np.ascontiguousarray(np.asarray(freqs_cos, np.float32).T).astype(BF16)
    sin_b = np.ascontiguousarray(np.asarray(freqs_sin, np.float32).T).astype(BF16)
    ones_b = np.ones((128, 128), BF16)
    Wq = np.asarray(Wq, np.float32)
    Wk = np.asarray(Wk, np.float32)
    Wv = np.asarray(Wv, np.float32)
    Wo = np.asarray(Wo, np.float32)

    in_maps = []
    for c in range(NCORES):
        pcols = np.concatenate([(2 * c + j) * HD + _EVOD for j in range(HP)])
        ncols = slice(2 * c * HD, (2 * c + HP) * HD)
        in_maps.append({
            "xT": xTb,
            "wq": np.ascontiguousarray(Wq[:, pcols]).astype(BF16),
            "wk": np.ascontiguousarray(Wk[:, pcols]).astype(BF16),
            "wv": np.ascontiguousarray(Wv[:, ncols]).astype(BF16),
            "wo": np.ascontiguousarray(Wo[ncols, :]).astype(BF16),
            "cosT": cos_b,
            "sinT": sin_b,
            "ones": ones_b,
        })

    res = run_bass_kernel_spmd(
        nc, in_maps, core_ids=list(range(NCORES)), trace=_TRACE,
    )
    _LAST_EXEC_NS = res.exec_time_ns
    _LAST_RES = res

    acc = np.zeros((B * S, D), np.float32)
    for c in range(NCORES):
        acc += res.results[c]["out"].astype(np.float32)
    return acc.reshape(B, S, D)


# revision 13
# speedup vs baseline: 1.6114x; 1.0477x over previous
"""Distributed Bass kernel for causal multi-head attention with RoPE.

Problem: B=2, S=2048, D=2048, H=16, HD=128 (nn_Attention_85315230368481).

Sharding: tensor-parallel over heads. Core c owns heads (2c, 2c+1); x is
replicated. Each core projects Q/K/V for its 2 heads over the full
sequence, applies RoPE, computes causal attention, and multiplies by its
row-slice of Wo, producing a partial [B*S, D] output. The host sums the
8 partials (the output is sum-sharded over head groups) — no device
collectives at all.

Performance structure:
 - Causal tile skipping: per 512-query chunk qc only key tiles
   kt <= 4*qc+3 are computed (~37% of score/attnV/exp work skipped).
   The causal mask inside the diagonal band is applied by a single
   gpsimd.affine_select per tile (no mask tensor, no DVE adds).
 - Softmax denominator via an all-ones [128,128] stationary matmul that
   also broadcasts the row sums across all partitions for free; the
   divide is folded into the PSUM-evacuation scalar_tensor_tensor.
 - Everything bf16 into the PE (fp32 PSUM accumulate); RoPE runs in
   bf16 split across DVE and GpSimd.
 - Batches processed one at a time so x^T stays SBUF-resident per batch;
   batch 1's x/weight DMAs overlap batch 0's attention.
"""

import sys

import ml_dtypes
import numpy as np

if "/opt/trn_rl_repo" not in sys.path:
    sys.path.insert(0, "/opt/trn_rl_repo")

B, S, D, H = 2, 2048, 2048, 16
HD = D // H            # 128
NCORES = 8
HP = H // NCORES       # 2 heads per core
NCH = D // 128         # 16 contraction chunks
NP = S // 512          # 4 row pieces per batch (projection)
NQC = S // 512         # 4 query chunks per batch (attention)
NKT = S // 128         # 16 key tiles per batch
SCALE = 1.0 / float(np.sqrt(HD))
BF16 = ml_dtypes.bfloat16

_GRAPH = None
_TRACE = False
_LAST_EXEC_NS = None
_LAST_RES = None


def _build_graph():
    import concourse.mybir as mybir
    from concourse import bacc, tile

    f32 = mybir.dt.float32
    bf = mybir.dt.bfloat16
    Exp = mybir.ActivationFunctionType.Exp

    nc = bacc.Bacc("TRN2", target_bir_lowering=False, num_devices=NCORES)

    xT = nc.declare_dram_parameter("xT", [D, B * S], bf, isOutput=False)
    wq = nc.declare_dram_parameter("wq", [D, HP * HD], bf, isOutput=False)
    wk = nc.declare_dram_parameter("wk", [D, HP * HD], bf, isOutput=False)
    wv = nc.declare_dram_parameter("wv", [D, HP * HD], bf, isOutput=False)
    wo = nc.declare_dram_parameter("wo", [HP * HD, D], bf, isOutput=False)
    cosT = nc.declare_dram_parameter("cosT", [64, S], f32, isOutput=False)
    sinT = nc.declare_dram_parameter("sinT", [64, S], f32, isOutput=False)
    onesd = nc.declare_dram_parameter("ones", [128, 128], bf, isOutput=False)
    out = nc.declare_dram_parameter("out", [B * S, D], bf, isOutput=True)

    with nc.allow_low_precision(reason="bf16 matmul inputs; fp32 accumulate"), \
         tile.TileContext(nc) as tc:
        with (
            tc.tile_pool(name="const", bufs=1) as constp,
            tc.tile_pool(name="xpool", bufs=1) as xpool,
            tc.tile_pool(name="kqv", bufs=2) as kqvp,
            tc.tile_pool(name="wstream", bufs=2) as wpool,
            tc.tile_pool(name="ropetmp", bufs=4) as rtp,
            tc.tile_pool(name="extiles", bufs=6) as expool,
            tc.tile_pool(name="rpool", bufs=2) as rpool,
            tc.tile_pool(name="ostage", bufs=2) as ostag,
            tc.tile_pool(name="projps", bufs=2, space="PSUM") as projps,
            tc.tile_pool(name="scps", bufs=2, space="PSUM") as scps,
            tc.tile_pool(name="attps", bufs=2, space="PSUM") as attps,
            tc.tile_pool(name="denps", bufs=2, space="PSUM") as denps,
        ):
            # cos/sin duplicated on partitions 0-63 and 64-127 so RoPE can
            # multiply the full [te; to] projection PSUM tile in one op
            cos_sb = constp.tile([128, S], f32)
            sin_sb = constp.tile([128, S], f32)
            ones_sb = constp.tile([128, 128], bf)
            nc.sync.dma_start(out=cos_sb[0:64, :], in_=cosT[:, :])
            nc.sync.dma_start(out=cos_sb[64:128, :], in_=cosT[:, :])
            nc.sync.dma_start(out=sin_sb[0:64, :], in_=sinT[:, :])
            nc.sync.dma_start(out=sin_sb[64:128, :], in_=sinT[:, :])
            nc.sync.dma_start(out=ones_sb[:], in_=onesd[:, :])

            for b in range(B):
                # ---- load x^T for this batch ----
                xts = xpool.tile([128, NCH * S], bf, tag="xts", name="xts")
                for c in range(NCH):
                    nc.sync.dma_start(
                        out=xts[:, c * S:(c + 1) * S],
                        in_=xT[c * 128:(c + 1) * 128, b * S:(b + 1) * S],
                    )

                ktsb = kqvp.tile([128, HP * S], bf, tag="kt", name="ktsb")
                qasb = kqvp.tile([128, HP * S], bf, tag="qa", name="qasb")
                vsb = kqvp.tile([128, NKT * HP * HD], bf, tag="v", name="vsb")

                # ---- K / Q projections + RoPE ----
                for wdram, dst in ((wk, ktsb), (wq, qasb)):
                    wsb = wpool.tile([128, NCH * 256], bf, tag="w", name="wsb")
                    for c in range(NCH):
                        nc.sync.dma_start(
                            out=wsb[:, c * 256:(c + 1) * 256],
                            in_=wdram[c * 128:(c + 1) * 128, :],
                        )
                    for piece in range(NP):
                        for ht in range(HP):
                            pp = projps.tile([128, 512], f32, tag="pp",
                                             name="pptile")
                            for c in range(NCH):
                                nc.tensor.matmul(
                                    pp[:],
                                    lhsT=wsb[:, c * 256 + ht * 128:
                                             c * 256 + (ht + 1) * 128],
                                    rhs=xts[:, c * S + piece * 512:
                                            c * S + piece * 512 + 512],
                                    start=(c == 0), stop=(c == NCH - 1),
                                )
                            # RoPE straight off the PSUM tile (PSUM-source
                            # DVE ops dodge the SBUF-source slowdown and the
                            # SBUF same-start-partition rule):
                            #   P1 = [te*c ; to*c]   (one full-height mul)
                            #   P2 = [to*s ; te*s]   (two half-height muls,
                            #                         halves swapped)
                            #   re = P1[lo] - P2[lo];  im = P1[hi] + P2[hi]
                            pc = slice(piece * 512, piece * 512 + 512)
                            dc = slice(ht * S + piece * 512,
                                       ht * S + piece * 512 + 512)
                            p1 = rtp.tile([128, 512], f32, tag="p1", name="p1")
                            p2 = rtp.tile([128, 512], f32, tag="p2", name="p2")
                            nc.vector.tensor_mul(p1[:], pp[:], cos_sb[:, pc])
                            nc.vector.tensor_mul(
                                p2[0:64, :], pp[64:128, :], sin_sb[64:128, pc])
                            nc.vector.tensor_mul(
                                p2[64:128, :], pp[0:64, :], sin_sb[0:64, pc])
                            nc.gpsimd.tensor_sub(
                                dst[0:64, dc], p1[0:64, :], p2[0:64, :])
                            nc.gpsimd.tensor_add(
                                dst[64:128, dc], p1[64:128, :], p2[64:128, :])

                # ---- V projection (natural layout) ----
                wsb = wpool.tile([128, NCH * 256], bf, tag="w", name="wsb")
                for c in range(NCH):
                    nc.sync.dma_start(
                        out=wsb[:, c * 256:(c + 1) * 256],
                        in_=wv[c * 128:(c + 1) * 128, :],
                    )
                for rt in range(NKT):
                    vp = denps.tile([128, 512], f32, tag="den", name="dentile")
                    for c in range(NCH):
                        nc.tensor.matmul(
                            vp[:, 0:256],
                            lhsT=xts[:, c * S + rt * 128:c * S + (rt + 1) * 128],
                            rhs=wsb[:, c * 256:(c + 1) * 256],
                            start=(c == 0), stop=(c == NCH - 1),
                        )
                    nc.vector.tensor_copy(vsb[:, rt * 256:(rt + 1) * 256],
                                          vp[:, 0:256])

                # ---- attention per head ----
                for h in range(HP):
                    for qc in range(NQC):
                        nkt = 4 * qc + 4          # live key tiles
                        att = attps.tile([128, 512], f32, tag="att",
                                         name="atttile")
                        den = denps.tile([128, 512], f32, tag="den",
                                         name="dentile")
                        for kt in range(nkt):
                            sc = scps.tile([128, 512], f32, tag="sc",
                                           name="sctile")
                            nc.tensor.matmul(
                                sc[:],
                                lhsT=ktsb[:, h * S + kt * 128:
                                          h * S + (kt + 1) * 128],
                                rhs=qasb[:, h * S + qc * 512:
                                         h * S + qc * 512 + 512],
                                start=True, stop=True,
                            )
                            ex = expool.tile([128, 512], bf, tag="ex",
                                             name="ex")
                            nc.scalar.activation(ex[:], sc[:], Exp,
                                                 scale=SCALE)
                            if kt >= 4 * qc:
                                # diagonal band: zero where key > query
                                nc.gpsimd.affine_select(
                                    out=ex[:], in_=ex[:],
                                    pattern=[[1, 512]],
                                    compare_op=mybir.AluOpType.is_ge,
                                    fill=0.0,
                                    base=qc * 512 - kt * 128,
                                    channel_multiplier=-1,
                                )
                            nc.tensor.matmul(
                                den[:], lhsT=ones_sb[:, :], rhs=ex[:],
                                start=(kt == 0), stop=(kt == nkt - 1),
                            )
                            nc.tensor.matmul(
                                att[:],
                                lhsT=vsb[:, kt * 256 + h * 128:
                                         kt * 256 + (h + 1) * 128],
                                rhs=ex[:],
                                start=(kt == 0), stop=(kt == nkt - 1),
                            )
                        rsb = rpool.tile([128, 512], f32, tag="rc", name="rsb")
                        nc.vector.reciprocal_approx_fast(rsb[:], den[:])
                        nc.vector.scalar_tensor_tensor(
                            out=qasb[:, h * S + qc * 512:
                                     h * S + qc * 512 + 512],
                            in0=att[:], scalar=1.0, in1=rsb[:],
                            op0=mybir.AluOpType.mult,
                            op1=mybir.AluOpType.mult,
                        )

                # ---- output projection (partial over this core's heads) ----
                wosb = wpool.tile([128, HP * D], bf, tag="wo", name="wosb")
                for ht in range(HP):
                    nc.sync.dma_start(
                        out=wosb[:, ht * D:(ht + 1) * D],
                        in_=wo[ht * 128:(ht + 1) * 128, :],
                    )
                for qt in range(NKT):
                    osb = ostag.tile([128, D], bf, tag="o", name="osb")
                    for ncol in range(4):
                        op = projps.tile([128, 512], f32, tag="pp",
                                         name="pptile")
                        for ht in range(HP):
                            nc.tensor.matmul(
                                op[:],
                                lhsT=qasb[:, ht * S + qt * 128:
                                          ht * S + (qt + 1) * 128],
                                rhs=wosb[:, ht * D + ncol * 512:
                                         ht * D + ncol * 512 + 512],
                                start=(ht == 0), stop=(ht == HP - 1),
                            )
                        oslice = osb[:, ncol * 512:(ncol + 1) * 512]
                        if ncol % 2 == 0:
                            nc.scalar.copy(oslice, op[:])
                        else:
                            nc.vector.tensor_copy(oslice, op[:])
                    nc.sync.dma_start(
                        out=out[b * S + qt * 128:b * S + (qt + 1) * 128, :],
                        in_=osb[:],
                    )

    nc.compile()
    return nc


def _get_graph():
    global _GRAPH
    if _GRAPH is None:
        _GRAPH = _build_graph()
    return _GRAPH


# per-head column permutation: even dims then odd dims (RoPE partition trick)
_EVOD = np.concatenate([np.arange(0, HD, 2), np.arange(1, HD, 2)])


def kernel(x, Wq, Wk, Wv, Wo, freqs_cos, freqs_sin, mask):
    global _LAST_EXEC_NS, _LAST_RES
    from concourse.bass_utils import run_bass_kernel_spmd

    nc = _get_graph()

    x = np.asarray(x, np.float32).reshape(B * S, D)
    xTb = np.ascontiguousarray(x.T).astype(BF16)
    cos_b = np.ascontiguousarray(np.asarray(freqs_cos, np.float32).T)
    sin_b = np.ascontiguousarray(np.asarray(freqs_sin, np.float32).T)
    ones_b = np.ones((128, 128), BF16)
    Wq = np.asarray(Wq, np.float32)
    Wk = np.asarray(Wk, np.float32)
    Wv = np.asarray(Wv, np.float32)
    Wo = np.asarray(Wo, np.float32)

    in_maps = []
    for c in range(NCORES):
        pcols = np.concatenate([(2 * c + j) * HD + _EVOD for j in range(HP)])
        ncols = slice(2 * c * HD, (2 * c + HP) * HD)
        in_maps.append({
            "xT": xTb,
            "wq": np.ascontiguousarray(Wq[:, pcols]).astype(BF16),
            "wk": np.ascontiguousarray(Wk[:, pcols]).astype(BF16),
            "wv": np.ascontiguousarray(Wv[:, ncols]).astype(BF16),
            "wo": np.ascontiguousarray(Wo[ncols, :]).astype(BF16),
            "cosT": cos_b,
            "sinT": sin_b,
            "ones": ones_b,
        })

    res = run_bass_kernel_spmd(
        nc, in_maps, core_ids=list(range(NCORES)), trace=_TRACE,
    )
    _LAST_EXEC_NS = res.exec_time_ns
    _LAST_RES = res

    acc = np.zeros((B * S, D), np.float32)
    for c in range(NCORES):
        acc += res.results[c]["out"].astype(np.float32)
    return acc.reshape(B, S, D)
